# revision 2
# baseline (speedup 1.0000x reference)
"""Trainium2 Bass kernel for nn_LongformerPersonalizedClsHead (MoE routing head).

Reference computation (B=256, S=512, H=768, U=100, L=2):
    x  = hidden_states[:, 0, :]                      # [B, H]  (CLS token only)
    z  = sum_u mask[b,u] * (x @ dense_W[u]) + mask @ dense_b
    h  = tanh(z)
    out= sum_u mask[b,u] * (h @ out_proj_W[u]) + mask @ out_proj_b   # [B, L]

Strategy: expert-parallel over 8 NeuronCores (13 expert slots/core, U padded
100->104). Each core streams its 13 [768,768] fp32 expert matrices from HBM
(~29 MB/core, the memory roofline), computes per-expert y_u = x @ W_u on the
tensor engine (float32r feed mode, full rate), combines with mask weights on
ACT+DVE into a partial z, AllReduces z across the 8 cores on-device, applies
tanh, and computes its experts' share of the tiny output projection. Host sums
the 8 [256,2] partials (the unshard step for expert sharding).
"""
import numpy as np

B, S, H, U, L = 256, 512, 768, 100, 2
N_CORES = 8
UPC = 13          # expert slots per core (8*13 = 104 >= 100, zero-padded)
UPAD = N_CORES * UPC
NB = B // 128     # 2 batch tiles
NH = H // 128     # 6 contraction chunks
NKS = 2           # two 384-wide output spans
KS = H // NKS
L2W = L * UPC     # 26 columns of the layer-2 moving operand

_RUNNER = None


def _build_nc():
    import concourse.bacc as bacc
    import concourse.mybir as mybir
    import concourse.tile as tile
    from concourse.masks import make_identity

    f32 = mybir.dt.float32
    f32r = mybir.dt.float32r

    nc = bacc.Bacc("TRN2", target_bir_lowering=False)

    xT = nc.dram_tensor("xT", [H, B], f32r, kind="ExternalInput")
    w = nc.dram_tensor("w", [UPC, H, H], f32r, kind="ExternalInput")
    mask = nc.dram_tensor("mask", [B, UPC], f32, kind="ExternalInput")
    maskT = nc.dram_tensor("maskT", [UPC, B], f32r, kind="ExternalInput")
    db = nc.dram_tensor("db", [UPC, H], f32r, kind="ExternalInput")
    woT = nc.dram_tensor("woT", [H, L2W], f32, kind="ExternalInput")
    bo = nc.dram_tensor("bo", [1, L2W], f32, kind="ExternalInput")
    mask_rep = nc.dram_tensor("mask_rep", [B, L2W], f32, kind="ExternalInput")
    o = nc.dram_tensor("o", [B, L], f32, kind="ExternalOutput")

    z_part = nc.dram_tensor("z_part", [B, H], f32)
    z_red = nc.dram_tensor("z_red", [B, H], f32, addr_space="Shared")

    with tile.TileContext(nc) as tc:
        with (
            tc.tile_pool(name="const", bufs=1) as cpool,
            tc.tile_pool(name="wpool", bufs=3) as wpool,
            tc.tile_pool(name="tmp", bufs=6) as tpool,
            tc.tile_pool(name="py", bufs=6, space="PSUM") as psum_y,
            tc.tile_pool(name="pmisc", bufs=1, space="PSUM") as psum_m,
        ):
            # --- resident inputs ---
            xT_sb = cpool.tile([128, NH, B], f32r, tag="xT")
            nc.sync.dma_start(xT_sb[:], xT.rearrange("(c p) b -> p c b", p=128))
            mask_sb = cpool.tile([128, NB, UPC], f32, tag="mask")
            nc.sync.dma_start(mask_sb[:], mask.rearrange("(nb p) u -> p nb u", p=128))
            maskT_sb = cpool.tile([UPC, B], f32r, tag="maskT")
            nc.sync.dma_start(maskT_sb[:], maskT[:])
            db_sb = cpool.tile([UPC, H], f32r, tag="db")
            nc.sync.dma_start(db_sb[:], db[:])
            woT_sb = cpool.tile([128, NH, L2W], f32, tag="woT")
            nc.sync.dma_start(woT_sb[:], woT.rearrange("(c p) l -> p c l", p=128))
            bo_sb = cpool.tile([1, L2W], f32, tag="bo")
            nc.sync.dma_start(bo_sb[:], bo[:])
            mrep_sb = cpool.tile([128, NB, L2W], f32, tag="mrep")
            nc.sync.dma_start(mrep_sb[:], mask_rep.rearrange("(nb p) l -> p nb l", p=128))
            ident = cpool.tile([128, 128], f32, tag="ident")
            make_identity(nc, ident[:])
            ones = cpool.tile([1, 128], f32, tag="ones")
            nc.vector.memset(ones[:], 1.0)

            z_sb = cpool.tile([128, NB, H], f32, tag="z")

            # --- layer-1 bias partial seeds z ---
            for b in range(NB):
                for ks in range(NKS):
                    acc = psum_y.tile([128, KS], f32, tag="y")
                    nc.tensor.matmul(
                        acc[:],
                        maskT_sb[:, b * 128:(b + 1) * 128],
                        db_sb[:, ks * KS:(ks + 1) * KS],
                        start=True, stop=True,
                    )
                    nc.scalar.copy(z_sb[:, b, ks * KS:(ks + 1) * KS], acc[:])

            # --- expert stream: z += mask[:,u] * (x @ W_u) ---
            for u in range(UPC):
                w_sb = wpool.tile([128, NH, H], f32r, tag="w")
                nc.sync.dma_start(w_sb[:], w[u].rearrange("(c p) k -> p c k", p=128))
                for b in range(NB):
                    for ks in range(NKS):
                        acc = psum_y.tile([128, KS], f32, tag="y")
                        for hc in range(NH):
                            nc.tensor.matmul(
                                acc[:],
                                xT_sb[:, hc, b * 128:(b + 1) * 128],
                                w_sb[:, hc, ks * KS:(ks + 1) * KS],
                                start=(hc == 0), stop=(hc == NH - 1),
                            )
                        tmp = tpool.tile([128, KS], f32, tag="tmp")
                        nc.scalar.mul(tmp[:], acc[:], mask_sb[:, b, u:u + 1])
                        zsl = z_sb[:, b, ks * KS:(ks + 1) * KS]
                        nc.vector.tensor_add(zsl, zsl, tmp[:])

            # --- AllReduce partial z across the 8 cores ---
            nc.sync.dma_start(z_part.rearrange("(nb p) k -> p nb k", p=128), z_sb[:])
            nc.gpsimd.collective_compute(
                "AllReduce",
                mybir.AluOpType.add,
                ins=[z_part[:]],
                outs=[z_red[:]],
                replica_groups=[list(range(N_CORES))],
            )

            # --- tanh + transpose h ---
            h_sb = cpool.tile([128, NB, H], f32, tag="h")
            zr_sb = cpool.tile([128, NB, H], f32, tag="zr")
            nc.sync.dma_start(zr_sb[:], z_red.rearrange("(nb p) k -> p nb k", p=128))
            for b in range(NB):
                nc.scalar.activation(
                    h_sb[:, b, :], zr_sb[:, b, :], mybir.ActivationFunctionType.Tanh
                )
            hT_sb = cpool.tile([128, NH, B], f32, tag="hT")
            for b in range(NB):
                for hc in range(NH):
                    tp = psum_m.tile([128, 128], f32, tag="tp")
                    nc.tensor.transpose(tp[:], h_sb[:, b, hc * 128:(hc + 1) * 128], ident[:])
                    nc.vector.tensor_copy(hT_sb[:, hc, b * 128:(b + 1) * 128], tp[:])

            # --- layer 2: Q = h @ woT (+ ones x bo seed), combine with mask ---
            o_sb = cpool.tile([128, NB, L], f32, tag="o")
            for b in range(NB):
                q = psum_m.tile([128, L2W], f32, tag="q")
                nc.tensor.matmul(q[:], ones[:], bo_sb[:], start=True, stop=False)
                for hc in range(NH):
                    nc.tensor.matmul(
                        q[:],
                        hT_sb[:, hc, b * 128:(b + 1) * 128],
                        woT_sb[:, hc, :],
                        start=False, stop=(hc == NH - 1),
                    )
                p = tpool.tile([128, L2W], f32, tag="p2")
                nc.vector.tensor_mul(p[:], q[:], mrep_sb[:, b, :])
                nc.vector.reduce_sum(
                    o_sb[:, b, :],
                    p[:].rearrange("p (l u) -> p l u", u=UPC),
                    axis=mybir.AxisListType.X,
                )
            nc.sync.dma_start(o.rearrange("(nb p) l -> p nb l", p=128), o_sb[:])

    nc.finalize()
    return nc


class _SpmdRunner:
    """Cached PJRT SPMD runner (mirrors concourse.bass2jax.run_bass_via_pjrt,
    but keeps the jitted callable alive so repeat calls don't re-trace)."""

    def __init__(self, nc, n_cores):
        import jax
        import concourse.mybir as mybir
        from concourse.bass2jax import (
            _bass_exec_p, install_neuronx_cc_hook, partition_id_tensor,
        )
        from jax.sharding import Mesh, PartitionSpec, NamedSharding
        try:
            from jax.experimental.shard_map import shard_map
        except ImportError:
            from jax.shard_map import shard_map

        install_neuronx_cc_hook()
        self.jax = jax
        self.nc = nc
        self.n_cores = n_cores

        in_names, out_names, out_avals, zero_outs = [], [], [], []
        partition_name = nc.partition_id_tensor.name if nc.partition_id_tensor else None
        dbg_name = None
        if nc.dbg_addr is not None:
            assert not nc.dbg_callbacks
            dbg_name = nc.dbg_addr.name
        for alloc in nc.m.functions[0].allocations:
            if not isinstance(alloc, mybir.MemoryLocationSet):
                continue
            name = alloc.memorylocations[0].name
            if alloc.kind == "ExternalInput":
                if name not in (partition_name, dbg_name):
                    in_names.append(name)
            elif alloc.kind == "ExternalOutput":
                out_names.append(name)
                shape = tuple(alloc.tensor_shape)
                dtype = mybir.dt.np(alloc.dtype)
                out_avals.append(jax.core.ShapedArray(shape, dtype))
                zero_outs.append(np.zeros(shape, dtype))

        self.in_names = list(in_names)
        self.out_names = list(out_names)
        self.zero_outs = zero_outs

        n_params = len(in_names)
        bound_names = list(in_names) + list(out_names)
        if dbg_name is not None:
            bound_names.append(dbg_name)
        if partition_name is not None:
            bound_names.append(partition_name)

        def _body(*args):
            operands = list(args)
            if dbg_name is not None:
                operands.append(jax.numpy.zeros((1, 2), jax.numpy.uint32))
            if partition_name is not None:
                operands.append(partition_id_tensor())
            outs = _bass_exec_p.bind(
                *operands,
                out_avals=tuple(out_avals),
                in_names=tuple(bound_names),
                out_names=tuple(self.out_names),
                lowering_input_output_aliases=(),
                sim_require_finite=True,
                sim_require_nnan=True,
                nc=nc,
            )
            return tuple(outs)

        devices = jax.devices()[:n_cores]
        assert len(devices) == n_cores, f"need {n_cores} cores, have {len(devices)}"
        self.mesh = Mesh(np.asarray(devices), ("core",))
        self.spec = PartitionSpec("core")
        self.sharding = NamedSharding(self.mesh, self.spec)
        n_args = n_params + len(out_names)
        self._jit = jax.jit(
            shard_map(
                _body,
                mesh=self.mesh,
                in_specs=(self.spec,) * n_args,
                out_specs=(self.spec,) * len(out_names),
                check_rep=False,
            ),
            keep_unused=True,
        )

    def put(self, in_maps):
        args = []
        for name in self.in_names:
            arrs = [np.asarray(in_maps[c][name]) for c in range(self.n_cores)]
            args.append(np.concatenate(arrs, axis=0))
        for z in self.zero_outs:
            args.append(np.concatenate([z] * self.n_cores, axis=0))
        return [self.jax.device_put(a, self.sharding) for a in args]

    def run_device(self, device_args):
        return self._jit(*device_args)

    def run(self, in_maps):
        outs = self._jit(*self.put(in_maps))
        np_outs = [np.asarray(o) for o in outs]
        results = []
        for c in range(self.n_cores):
            d = {}
            for i, name in enumerate(self.out_names):
                full = np_outs[i]
                per = full.shape[0] // self.n_cores
                d[name] = full[c * per:(c + 1) * per]
            results.append(d)
        return results


def _get_runner():
    global _RUNNER
    if _RUNNER is None:
        _RUNNER = _SpmdRunner(_build_nc(), N_CORES)
    return _RUNNER


def _prep_in_maps(hidden_states, user_mask, dense_W, dense_b, out_proj_W, out_proj_b):
    x = np.ascontiguousarray(hidden_states[:, 0, :], dtype=np.float32)   # [B, H]
    xT = np.ascontiguousarray(x.T)                                       # [H, B]

    # pad experts to 104
    pad = UPAD - U
    mask_p = np.concatenate([user_mask, np.zeros((B, pad), np.float32)], axis=1)
    w_p = np.concatenate([dense_W, np.zeros((pad, H, H), np.float32)], axis=0)
    db_p = np.concatenate([dense_b, np.zeros((pad, H), np.float32)], axis=0)
    wo_p = np.concatenate([out_proj_W, np.zeros((pad, H, L), np.float32)], axis=0)
    bo_p = np.concatenate([out_proj_b, np.zeros((pad, L), np.float32)], axis=0)

    in_maps = []
    for c in range(N_CORES):
        sl = slice(c * UPC, (c + 1) * UPC)
        m = np.ascontiguousarray(mask_p[:, sl])                 # [B, UPC]
        woT = np.ascontiguousarray(
            wo_p[sl].transpose(1, 2, 0).reshape(H, L2W))        # [H, (l,u)]
        bo_flat = np.ascontiguousarray(
            bo_p[sl].T.reshape(1, L2W))                         # [1, (l,u)]
        in_maps.append({
            "xT": xT,
            "w": np.ascontiguousarray(w_p[sl]),
            "mask": m,
            "maskT": np.ascontiguousarray(m.T),
            "db": np.ascontiguousarray(db_p[sl]),
            "woT": woT,
            "bo": bo_flat,
            "mask_rep": np.ascontiguousarray(np.concatenate([m, m], axis=1)),
        })
    return in_maps


def kernel(hidden_states, user_mask, dense_W, dense_b, out_proj_W, out_proj_b):
    hidden_states = np.asarray(hidden_states, dtype=np.float32)
    user_mask = np.asarray(user_mask, dtype=np.float32)
    dense_W = np.asarray(dense_W, dtype=np.float32)
    dense_b = np.asarray(dense_b, dtype=np.float32)
    out_proj_W = np.asarray(out_proj_W, dtype=np.float32)
    out_proj_b = np.asarray(out_proj_b, dtype=np.float32)

    runner = _get_runner()
    in_maps = _prep_in_maps(hidden_states, user_mask, dense_W, dense_b,
                            out_proj_W, out_proj_b)
    results = runner.run(in_maps)
    out = np.zeros((B, L), np.float32)
    for c in range(N_CORES):
        out += results[c]["o"]
    return out
